# revision 18
# baseline (speedup 1.0000x reference)
"""Causal self-attention (B=16, T=1024, C=768, NH=12) on 8 trn2 NeuronCores.

Data-parallel over the batch dim (2 batches per core, no collectives).
All matmuls run in bf16 (fp32 PSUM accumulation); inputs are cast to bf16
and x is pre-transposed on the host. Weights are repacked on the host so
each SBUF-resident tensor loads with large per-partition DMA packets.

Per batch, layouts keep the softmax reduction on the PSUM free dimension:

  xT   [C, T]       host-pretransposed, DMA'd directly
  qT   [768, T]     = W_q-chunk-lhsT @ xT  (2 heads per 128-row tile)
  kT   [768, T]     likewise
  v    [T, 768]     = xT-chunk-lhsT @ W_v  (+ a ones column per head so
                      attn@v also yields the softmax denominator Z)
  scoresT[k, q]     = kT-slice-lhsT @ qT-slice  (K=64; the two heads of a
                      pair run concurrently in array rows 0-63 / 64-127)
  attnT             = exp(scoresT / 8) on ACT, diagonal tiles masked on DVE
  avT [65, q]       = v_aug-lhsT @ attnT, accumulated over k chunks in PSUM
  attOutT[d, q]     = avT[0:64] * (1/Z broadcast via K=1 matmul)
  y    [T, C]       = attOutT-chunk-lhsT @ W_proj + bias   (bf16 out)

The projection work of the *other* batch (v-proj of b+1, y-proj of b-1) is
interleaved into each batch's attention phase so the tensor engine's
activity monitor (HAM) keeps the PE clock at full rate.
"""
import numpy as np

B, T, C = 16, 1024, 768
NH, HD = 12, 64
NCORES = 8
BPC = B // NCORES          # batches per core
NP = NH // 2               # head pairs (128-row o-tiles)
NT = T // 128              # 128-row seq tiles
NST = T // 512             # 512-col q supertiles
NKC = C // 128             # 128-row contraction chunks of C

_CACHE = {}


def _build_nc():
    import concourse.bacc as bacc
    import concourse.mybir as mybir
    import concourse.tile as tile

    F32 = mybir.dt.float32
    BF16 = mybir.dt.bfloat16
    EXP = mybir.ActivationFunctionType.Exp

    nc = bacc.Bacc("TRN2", target_bir_lowering=False)

    # xt rows are C (chunked 128/partition-tile); cols are [b0 T | b1 T]
    xt_in = nc.dram_tensor("xt", [C, BPC * T], BF16, kind="ExternalInput")
    # wa packed [128, NKC*3C]: col block kc holds wa[128*kc:128*kc+128, :]
    wa = nc.dram_tensor("wa", [128, NKC * 3 * C], BF16, kind="ExternalInput")
    # wp packed [128, NP*C]: col block hp holds wp[128*hp:128*hp+128, :]
    wp = nc.dram_tensor("wp", [128, NP * C], BF16, kind="ExternalInput")
    bqk = nc.dram_tensor("bqk", [128, 2 * NP], F32, kind="ExternalInput")
    bv = nc.dram_tensor("bv", [128, C], BF16, kind="ExternalInput")
    bp = nc.dram_tensor("bp", [128, C], BF16, kind="ExternalInput")
    mask = nc.dram_tensor("mask", [128, 256], BF16, kind="ExternalInput")
    ones = nc.dram_tensor("ones", [128, 64], BF16, kind="ExternalInput")
    y_out = nc.dram_tensor("y", [BPC, T, C], BF16, kind="ExternalOutput")

    with tile.TileContext(nc) as tc:
        with (
            tc.tile_pool(name="consts", bufs=1) as consts,
            tc.tile_pool(name="sb", bufs=1) as sb,
            tc.tile_pool(name="ps", bufs=1, space="PSUM") as ps,
        ):
            # ---- resident weights / constants (split across queues,
            # ordered so the first compute's inputs arrive first) --------
            wa_sb = consts.tile([128, NKC * 3 * C], BF16, tag="wa")
            wa_t = [wa_sb[:, 3 * C * kc:3 * C * (kc + 1)] for kc in range(NKC)]

            def wa_block(ap, chunks, lo, hi):
                v = ap[:, 3 * C * chunks.start:3 * C * chunks.stop]
                v = v.rearrange("p (c w) -> p c w", w=3 * C)
                return v[:, :, lo:hi]

            # qk columns of wa first (gate the first qkT chains)
            nc.gpsimd.dma_start(wa_block(wa_sb, slice(0, 2), 0, 2 * C),
                                wa_block(wa, slice(0, 2), 0, 2 * C))
            nc.scalar.dma_start(wa_block(wa_sb, slice(2, 4), 0, 2 * C),
                                wa_block(wa, slice(2, 4), 0, 2 * C))
            # xT: batch-0 columns first (gate everything)
            xT = [sb.tile([128, BPC * T], BF16, tag="xT", bufs=NKC,
                          name=f"xT{c}") for c in range(NKC)]
            for c in range(NKC):
                nc.sync.dma_start(
                    xT[c][:, 0:T], xt_in[128 * c:128 * c + 128, 0:T])
            nc.sync.dma_start(wa_block(wa_sb, slice(4, 6), 0, 2 * C),
                              wa_block(wa, slice(4, 6), 0, 2 * C))
            for c in range(NKC):
                nc.sync.dma_start(
                    xT[c][:, T:2 * T],
                    xt_in[128 * c:128 * c + 128, T:2 * T])
            # small consts first (gate bias adds / first masks)
            bqk_sb = consts.tile([128, 2 * NP], F32, tag="bqk")
            nc.scalar.dma_start(bqk_sb[:], bqk[:])
            bv_sb = consts.tile([128, C], BF16, tag="bv")
            nc.scalar.dma_start(bv_sb[:], bv[:])
            mask_sb = consts.tile([128, 256], BF16, tag="mask")
            nc.scalar.dma_start(mask_sb[:], mask[:])
            ones_sb = consts.tile([128, 64], BF16, tag="ones")
            nc.scalar.dma_start(ones_sb[:], ones[:])
            # v columns of wa next
            nc.gpsimd.dma_start(wa_block(wa_sb, slice(0, 3), 2 * C, 3 * C),
                                wa_block(wa, slice(0, 3), 2 * C, 3 * C))
            nc.scalar.dma_start(wa_block(wa_sb, slice(3, 6), 2 * C, 3 * C),
                                wa_block(wa, slice(3, 6), 2 * C, 3 * C))

            wp_sb = consts.tile([128, NP * C], BF16, tag="wp")
            nc.gpsimd.dma_start(wp_sb[:], wp[:])
            wp_t = [wp_sb[:, C * hp:C * (hp + 1)] for hp in range(NP)]
            bp_sb = consts.tile([128, C], BF16, tag="bp")
            nc.gpsimd.dma_start(bp_sb[:], bp[:])

            def x_of(b, kc, lo, hi):
                return xT[kc][:, T * b + lo:T * b + hi]

            def emit_qkT(b, hp):
                qT = sb.tile([128, T], BF16, tag="qT", bufs=4)
                kT = sb.tile([128, T], BF16, tag="kT", bufs=4)
                for dst, osel, bcol in ((qT, 0, hp), (kT, 1, NP + hp)):
                    obase = C * osel + 128 * hp
                    for st in range(NST):
                        pq = ps.tile([128, 512], F32, tag="pp", bufs=2)
                        for kc in range(NKC):
                            nc.tensor.matmul(
                                pq[:],
                                wa_t[kc][:, obase:obase + 128],
                                x_of(b, kc, 512 * st, 512 * st + 512),
                                start=(kc == 0), stop=(kc == NKC - 1))
                        nc.vector.tensor_scalar_add(
                            out=dst[:, 512 * st:512 * st + 512],
                            in0=pq[:],
                            scalar1=bqk_sb[:, bcol:bcol + 1])
                return qT, kT

            v_aug = {}

            def emit_vproj_chain(b, s, tt):
                pv = ps.tile([128, 512], F32, tag="pp", bufs=2)
                for kc in range(NKC):
                    nc.tensor.matmul(
                        pv[:, 0:384],
                        x_of(b, kc, 128 * tt, 128 * tt + 128),
                        wa_t[kc][:, 2 * C + 384 * s:2 * C + 384 * s + 384],
                        start=(kc == 0), stop=(kc == NKC - 1))
                va_v = v_aug[b][tt][:].rearrange(
                    "p (h c) -> p h c", c=65)[:, 6 * s:6 * s + 6, :]
                nc.vector.tensor_add(
                    out=va_v[:, :, 0:64],
                    in0=pv[:, 0:384].rearrange("p (h d) -> p h d", d=64),
                    in1=bv_sb[:, 384 * s:384 * s + 384].rearrange(
                        "p (h d) -> p h d", d=64))
                nc.vector.tensor_copy(va_v[:, :, 64], ones_sb[:, 0:6])

            def alloc_vaug(b):
                v_aug[b] = [sb.tile([128, NH * 65], BF16, tag="vaug",
                                    bufs=BPC * NT, name=f"vaug{b}_{i}")
                            for i in range(NT)]

            attOutT = {}

            def emit_yproj_tt(b, tt):
                y_sb = sb.tile([128, C], BF16, tag="ysb", bufs=3)
                for s in range(2):
                    py = ps.tile([128, 512], F32, tag="pp", bufs=2)
                    for hp in range(NP):
                        nc.tensor.matmul(
                            py[:, 0:384],
                            attOutT[b][hp][:, 128 * tt:128 * tt + 128],
                            wp_t[hp][:, 384 * s:384 * s + 384],
                            start=(hp == 0), stop=(hp == NP - 1))
                    nc.vector.tensor_add(
                        out=y_sb[:, 384 * s:384 * s + 384],
                        in0=py[:, 0:384],
                        in1=bp_sb[:, 384 * s:384 * s + 384])
                eng = (nc.sync, nc.gpsimd, nc.scalar)[tt % 3]
                eng.dma_start(y_out[b, 128 * tt:128 * tt + 128, :], y_sb[:])

            def attn_st(b, hp, st, qT, kT, aot, st2):
                nkc_av = 4 * (st + 1)
                av = [ps.tile([65, 512], F32, tag="avp", bufs=3,
                              name=f"av{i}") for i in range(2)]
                for kc in range(nkc_av):
                    off = 128 * kc - 512 * st
                    diag = off >= 0
                    start = max(off, 0)
                    n = 512 - start
                    sc_pair = []
                    at_pair = []
                    for par in range(2):
                        scp = ps.tile([128, 512], F32, tag="scp", bufs=3)
                        sc_pair.append(scp)
                        nc.tensor.matmul(
                            scp[:, 0:n],
                            kT[64 * par:64 * par + 64,
                               128 * kc:128 * kc + 128],
                            qT[64 * par:64 * par + 64,
                               512 * st + start:512 * st + 512],
                            start=True, stop=True)
                    for par in range(2):
                        at = sb.tile([128, 512], BF16, tag="attnT", bufs=8)
                        at_pair.append(at)
                        nc.scalar.activation(
                            at[:, 0:n], sc_pair[par][:, 0:n], EXP,
                            scale=0.125)
                        if diag:
                            nc.vector.tensor_mul(
                                out=at[:, 0:128], in0=at[:, 0:128],
                                in1=mask_sb[:, 0:128])
                    for par in range(2):
                        h = 2 * hp + par
                        nc.tensor.matmul(
                            av[par][0:65, start:512],
                            v_aug[b][kc][:, 65 * h:65 * h + 65],
                            at_pair[par][:, 0:n],
                            start=(kc == 0), stop=(kc == nkc_av - 1))
                # normalize: attOut = av[0:64] / Z  (Z = av row 64)
                rc_pair = []
                for par in range(2):
                    rc = sb.tile([128, 512], BF16, tag="rcp", bufs=3)
                    nc.scalar.copy(rc[64:65, :], av[par][64:65, :])
                    rc_pair.append(rc)
                bc_pair = []
                for par in range(2):
                    bcp = ps.tile([128, 512], F32, tag="pp", bufs=2)
                    nc.tensor.matmul(
                        bcp[0:64, :], ones_sb[64:65, 0:64],
                        rc_pair[par][64:65, :], start=True, stop=True)
                    bc_pair.append(bcp)
                rr_pair = []
                for par in range(2):
                    rcs = sb.tile([64, 512], F32, tag="rcf", bufs=3)
                    nc.vector.reciprocal_approx_fast(
                        out=rcs[:], in_=bc_pair[par][0:64, :])
                    rr_pair.append(rcs)
                nc.vector.tensor_mul(
                    out=aot[0:64, 512 * st:512 * st + 512],
                    in0=av[0][0:64, :], in1=rr_pair[0][:])
                nc.vector.tensor_mul(
                    out=st2[:, 512 * st:512 * st + 512],
                    in0=av[1][0:64, :], in1=rr_pair[1][:])

            # ================= main schedule ==========================
            alloc_vaug(0)
            for b in range(BPC):
                qk = [emit_qkT(b, 0)]
                if b == 0:
                    # first batch's v-proj runs up front (dense warm-up)
                    for s in range(2):
                        for tt in range(NT):
                            emit_vproj_chain(0, s, tt)
                    alloc_vaug(1)
                    filler = [(emit_vproj_chain, (1, s, tt))
                              for s in range(2) for tt in range(NT)]
                else:
                    filler = [(emit_yproj_tt, (0, tt)) for tt in range(NT)]
                fi = 0

                attOutT[b] = []
                for hp in range(NP):
                    if hp + 1 < NP:
                        qk.append(emit_qkT(b, hp + 1))
                    qT, kT = qk[hp]
                    aot = sb.tile([128, T], BF16, tag="aot", bufs=2 * NP + 1)
                    attOutT[b].append(aot)
                    st2 = sb.tile([64, T], BF16, tag="stg2", bufs=2)

                    attn_st(b, hp, 0, qT, kT, aot, st2)
                    # par=1 st0 half -> aot rows 64:128 right away so
                    # st0-dependent y tiles can start
                    nc.sync.dma_start(aot[64:128, 0:512], st2[:, 0:512])
                    # interleave dense projection work of the other batch
                    n_fill = (len(filler) * (hp + 1) * 2 + NP * 2 - 1) \
                        // (NP * 2)
                    while fi < min(n_fill, len(filler)):
                        f, a = filler[fi]
                        f(*a)
                        fi += 1
                    if b == 1 and hp == NP - 1:
                        # y tiles 0-3 need only the st0 halves of every
                        # head: overlap them with the last head's st1
                        for tt in range(NT // 2):
                            emit_yproj_tt(1, tt)
                    attn_st(b, hp, 1, qT, kT, aot, st2)
                    nc.scalar.dma_start(aot[64:128, 512:1024],
                                        st2[:, 512:1024])
                    n_fill = (len(filler) * ((hp + 1) * 2 + 1)
                              + NP * 2 - 1) // (NP * 2)
                    while fi < min(n_fill, len(filler)):
                        f, a = filler[fi]
                        f(*a)
                        fi += 1
                while fi < len(filler):
                    f, a = filler[fi]
                    f(*a)
                    fi += 1
            # final batch's y projection tail (st1-dependent tiles)
            for tt in range(NT // 2, NT):
                emit_yproj_tt(1, tt)

    nc.finalize()
    return nc


def _prep_const_inputs(W_attn, b_attn, W_proj, b_proj):
    import ml_dtypes
    BF = ml_dtypes.bfloat16
    W_attn = np.asarray(W_attn, dtype=np.float32)
    W_proj = np.asarray(W_proj, dtype=np.float32)
    b_attn = np.asarray(b_attn, dtype=np.float32)
    b_proj = np.asarray(b_proj, dtype=np.float32)
    # wa packed: col block kc = wa[128*kc:128*(kc+1), :]  -> [128, NKC*3C]
    wa = np.ascontiguousarray(
        W_attn.reshape(NKC, 128, 3 * C).transpose(1, 0, 2).reshape(
            128, NKC * 3 * C)).astype(BF)
    wp = np.ascontiguousarray(
        W_proj.reshape(NP, 128, C).transpose(1, 0, 2).reshape(
            128, NP * C)).astype(BF)
    bqk = np.ascontiguousarray(
        b_attn[:2 * C].reshape(2 * NP, 128).T).astype(np.float32)
    bv = np.ascontiguousarray(
        np.broadcast_to(b_attn[2 * C:], (128, C))).astype(BF)
    bp = np.ascontiguousarray(
        np.broadcast_to(b_proj, (128, C))).astype(BF)
    # diag-tile mask: mask[i, j] = 1 if j >= i  (q-col >= k-row)
    jj = np.arange(128)[None, :]
    ii = np.arange(128)[:, None]
    mask = np.concatenate([(jj >= ii)] * 2, axis=1).astype(BF)
    ones = np.ones((128, 64), dtype=BF)
    return {"wa": wa, "wp": wp, "bqk": bqk, "bv": bv, "bp": bp,
            "mask": mask, "ones": ones}


def _make_in_maps(x, W_attn, b_attn, W_proj, b_proj):
    import ml_dtypes
    BF = ml_dtypes.bfloat16
    consts = _prep_const_inputs(W_attn, b_attn, W_proj, b_proj)
    xb = np.asarray(x, dtype=np.float32).astype(BF)
    maps = []
    for c in range(NCORES):
        xc = xb[BPC * c:BPC * (c + 1)]            # [BPC, T, C]
        xtc = np.ascontiguousarray(
            xc.transpose(2, 0, 1).reshape(C, BPC * T))
        maps.append({"xt": xtc, **consts})
    return maps


def kernel(x, W_attn, b_attn, W_proj, b_proj):
    from concourse.bass_utils import run_bass_kernel_spmd

    if "nc" not in _CACHE:
        _CACHE["nc"] = _build_nc()
    nc = _CACHE["nc"]

    in_maps = _make_in_maps(x, W_attn, b_attn, W_proj, b_proj)
    res = run_bass_kernel_spmd(nc, in_maps, list(range(NCORES)))
    return np.concatenate(
        [np.asarray(r["y"], dtype=np.float32) for r in res.results], axis=0)


# revision 20
# speedup vs baseline: 1.0926x; 1.0926x over previous
"""Causal self-attention (B=16, T=1024, C=768, NH=12) on 8 trn2 NeuronCores.

Data-parallel over the batch dim (2 batches per core, no collectives).
All matmuls run in bf16 (fp32 PSUM accumulation); inputs are cast to bf16
and x is pre-transposed on the host. Weights are repacked on the host so
each SBUF-resident tensor loads with large per-partition DMA packets.

Per batch, layouts keep the softmax reduction on the PSUM free dimension:

  xT   [C, T]       host-pretransposed, DMA'd directly
  qT   [768, T]     = W_q-chunk-lhsT @ xT  (2 heads per 128-row tile)
  kT   [768, T]     likewise
  v    [T, 768]     = xT-chunk-lhsT @ W_v  (+ a ones column per head so
                      attn@v also yields the softmax denominator Z)
  scoresT[k, q]     = kT-slice-lhsT @ qT-slice  (K=64; the two heads of a
                      pair run concurrently in array rows 0-63 / 64-127)
  attnT             = exp(scoresT / 8) on ACT, diagonal tiles masked on DVE
  avT [65, q]       = v_aug-lhsT @ attnT, accumulated over k chunks in PSUM
  attOutT[d, q]     = avT[0:64] * (1/Z broadcast via K=1 matmul)
  y    [T, C]       = attOutT-chunk-lhsT @ W_proj + bias   (bf16 out)

The projection work of the *other* batch (v-proj of b+1, y-proj of b-1) is
interleaved into each batch's attention phase so the tensor engine's
activity monitor (HAM) keeps the PE clock at full rate.
"""
import numpy as np

B, T, C = 16, 1024, 768
NH, HD = 12, 64
NCORES = 8
BPC = B // NCORES          # batches per core
NP = NH // 2               # head pairs (128-row o-tiles)
NT = T // 128              # 128-row seq tiles
NST = T // 512             # 512-col q supertiles
NKC = C // 128             # 128-row contraction chunks of C

_CACHE = {}


def _build_nc():
    import concourse.bacc as bacc
    import concourse.mybir as mybir
    import concourse.tile as tile

    F32 = mybir.dt.float32
    BF16 = mybir.dt.bfloat16
    EXP = mybir.ActivationFunctionType.Exp

    nc = bacc.Bacc("TRN2", target_bir_lowering=False)

    # xt rows are C (chunked 128/partition-tile); cols are [b0 T | b1 T]
    xt_in = nc.dram_tensor("xt", [C, BPC * T], BF16, kind="ExternalInput")
    # wa packed [128, NKC*3C]: col block kc holds wa[128*kc:128*kc+128, :]
    wa = nc.dram_tensor("wa", [128, NKC * 3 * C], BF16, kind="ExternalInput")
    # wp packed [128, NP*C]: col block hp holds wp[128*hp:128*hp+128, :]
    wp = nc.dram_tensor("wp", [128, NP * C], BF16, kind="ExternalInput")
    bqk = nc.dram_tensor("bqk", [128, 2 * NP], F32, kind="ExternalInput")
    bv = nc.dram_tensor("bv", [128, C], BF16, kind="ExternalInput")
    bp = nc.dram_tensor("bp", [128, C], BF16, kind="ExternalInput")
    mask = nc.dram_tensor("mask", [128, 256], BF16, kind="ExternalInput")
    ones = nc.dram_tensor("ones", [128, 64], BF16, kind="ExternalInput")
    y_out = nc.dram_tensor("y", [BPC, T, C], BF16, kind="ExternalOutput")

    with tile.TileContext(nc) as tc:
        with (
            tc.tile_pool(name="consts", bufs=1) as consts,
            tc.tile_pool(name="sb", bufs=1) as sb,
            tc.tile_pool(name="ps", bufs=1, space="PSUM") as ps,
        ):
            # ---- resident weights / constants (split across queues,
            # ordered so the first compute's inputs arrive first) --------
            wa_sb = consts.tile([128, NKC * 3 * C], BF16, tag="wa")
            wa_t = [wa_sb[:, 3 * C * kc:3 * C * (kc + 1)] for kc in range(NKC)]

            def wa_block(ap, chunks, lo, hi):
                v = ap[:, 3 * C * chunks.start:3 * C * chunks.stop]
                v = v.rearrange("p (c w) -> p c w", w=3 * C)
                return v[:, :, lo:hi]

            # qk columns of wa first (gate the first qkT chains)
            nc.gpsimd.dma_start(wa_block(wa_sb, slice(0, 2), 0, 2 * C),
                                wa_block(wa, slice(0, 2), 0, 2 * C))
            nc.scalar.dma_start(wa_block(wa_sb, slice(2, 4), 0, 2 * C),
                                wa_block(wa, slice(2, 4), 0, 2 * C))
            # xT: batch-0 columns first (gate everything)
            xT = [sb.tile([128, BPC * T], BF16, tag="xT", bufs=NKC,
                          name=f"xT{c}") for c in range(NKC)]
            for c in range(NKC):
                nc.sync.dma_start(
                    xT[c][:, 0:T], xt_in[128 * c:128 * c + 128, 0:T])
            nc.sync.dma_start(wa_block(wa_sb, slice(4, 6), 0, 2 * C),
                              wa_block(wa, slice(4, 6), 0, 2 * C))
            for c in range(NKC):
                nc.sync.dma_start(
                    xT[c][:, T:2 * T],
                    xt_in[128 * c:128 * c + 128, T:2 * T])
            # small consts first (gate bias adds / first masks)
            bqk_sb = consts.tile([128, 2 * NP], F32, tag="bqk")
            nc.scalar.dma_start(bqk_sb[:], bqk[:])
            bv_sb = consts.tile([128, C], BF16, tag="bv")
            nc.scalar.dma_start(bv_sb[:], bv[:])
            mask_sb = consts.tile([128, 256], BF16, tag="mask")
            nc.scalar.dma_start(mask_sb[:], mask[:])
            ones_sb = consts.tile([128, 64], BF16, tag="ones")
            nc.scalar.dma_start(ones_sb[:], ones[:])
            # v columns of wa next
            nc.gpsimd.dma_start(wa_block(wa_sb, slice(0, 3), 2 * C, 3 * C),
                                wa_block(wa, slice(0, 3), 2 * C, 3 * C))
            nc.scalar.dma_start(wa_block(wa_sb, slice(3, 6), 2 * C, 3 * C),
                                wa_block(wa, slice(3, 6), 2 * C, 3 * C))

            wp_sb = consts.tile([128, NP * C], BF16, tag="wp")
            nc.gpsimd.dma_start(wp_sb[:], wp[:])
            wp_t = [wp_sb[:, C * hp:C * (hp + 1)] for hp in range(NP)]
            bp_sb = consts.tile([128, C], BF16, tag="bp")
            nc.gpsimd.dma_start(bp_sb[:], bp[:])

            def x_of(b, kc, lo, hi):
                return xT[kc][:, T * b + lo:T * b + hi]

            def emit_qkT(b, hp):
                qT = sb.tile([128, T], BF16, tag="qT", bufs=4)
                kT = sb.tile([128, T], BF16, tag="kT", bufs=4)
                for dst, osel, bcol in ((qT, 0, hp), (kT, 1, NP + hp)):
                    obase = C * osel + 128 * hp
                    for st in range(NST):
                        pq = ps.tile([128, 512], F32, tag="pp", bufs=2)
                        for kc in range(NKC):
                            nc.tensor.matmul(
                                pq[:],
                                wa_t[kc][:, obase:obase + 128],
                                x_of(b, kc, 512 * st, 512 * st + 512),
                                start=(kc == 0), stop=(kc == NKC - 1))
                        nc.vector.tensor_scalar_add(
                            out=dst[:, 512 * st:512 * st + 512],
                            in0=pq[:],
                            scalar1=bqk_sb[:, bcol:bcol + 1])
                return qT, kT

            v_aug = {}

            def emit_vproj_chain(b, s, tt):
                pv = ps.tile([128, 512], F32, tag="pp", bufs=2)
                for kc in range(NKC):
                    nc.tensor.matmul(
                        pv[:, 0:384],
                        x_of(b, kc, 128 * tt, 128 * tt + 128),
                        wa_t[kc][:, 2 * C + 384 * s:2 * C + 384 * s + 384],
                        start=(kc == 0), stop=(kc == NKC - 1))
                va_v = v_aug[b][tt][:].rearrange(
                    "p (h c) -> p h c", c=65)[:, 6 * s:6 * s + 6, :]
                nc.vector.tensor_add(
                    out=va_v[:, :, 0:64],
                    in0=pv[:, 0:384].rearrange("p (h d) -> p h d", d=64),
                    in1=bv_sb[:, 384 * s:384 * s + 384].rearrange(
                        "p (h d) -> p h d", d=64))
                nc.vector.tensor_copy(va_v[:, :, 64], ones_sb[:, 0:6])

            def alloc_vaug(b):
                v_aug[b] = [sb.tile([128, NH * 65], BF16, tag="vaug",
                                    bufs=BPC * NT, name=f"vaug{b}_{i}")
                            for i in range(NT)]

            attOutT = {}

            def emit_yproj_tt(b, tt):
                y_sb = sb.tile([128, C], BF16, tag="ysb", bufs=3)
                for s in range(2):
                    py = ps.tile([128, 512], F32, tag="pp", bufs=2)
                    for hp in range(NP):
                        nc.tensor.matmul(
                            py[:, 0:384],
                            attOutT[b][hp][:, 128 * tt:128 * tt + 128],
                            wp_t[hp][:, 384 * s:384 * s + 384],
                            start=(hp == 0), stop=(hp == NP - 1))
                    nc.vector.tensor_add(
                        out=y_sb[:, 384 * s:384 * s + 384],
                        in0=py[:, 0:384],
                        in1=bp_sb[:, 384 * s:384 * s + 384])
                eng = (nc.sync, nc.gpsimd, nc.scalar)[tt % 3]
                eng.dma_start(y_out[b, 128 * tt:128 * tt + 128, :], y_sb[:])

            def attn_st(b, hp, st, qT, kT, aot, st2):
                nkc_av = 4 * (st + 1)
                av = [ps.tile([65, 512], F32, tag="avp", bufs=2,
                              name=f"av{i}") for i in range(2)]
                for kc in range(nkc_av):
                    off = 128 * kc - 512 * st
                    diag = off >= 0
                    start = max(off, 0)
                    n = 512 - start
                    sc_pair = []
                    at_pair = []
                    for par in range(2):
                        scp = ps.tile([128, 512], F32, tag="scp", bufs=4)
                        sc_pair.append(scp)
                        nc.tensor.matmul(
                            scp[:, 0:n],
                            kT[64 * par:64 * par + 64,
                               128 * kc:128 * kc + 128],
                            qT[64 * par:64 * par + 64,
                               512 * st + start:512 * st + 512],
                            start=True, stop=True)
                    for par in range(2):
                        at = sb.tile([128, 512], BF16, tag="attnT", bufs=8)
                        at_pair.append(at)
                        nc.scalar.activation(
                            at[:, 0:n], sc_pair[par][:, 0:n], EXP,
                            scale=0.125)
                        if diag:
                            nc.vector.tensor_mul(
                                out=at[:, 0:128], in0=at[:, 0:128],
                                in1=mask_sb[:, 0:128])
                    for par in range(2):
                        h = 2 * hp + par
                        nc.tensor.matmul(
                            av[par][0:65, start:512],
                            v_aug[b][kc][:, 65 * h:65 * h + 65],
                            at_pair[par][:, 0:n],
                            start=(kc == 0), stop=(kc == nkc_av - 1))
                # normalize: attOut = av[0:64] / Z  (Z = av row 64)
                rc_pair = []
                for par in range(2):
                    rc = sb.tile([128, 512], BF16, tag="rcp", bufs=2)
                    nc.scalar.copy(rc[64:65, :], av[par][64:65, :])
                    rc_pair.append(rc)
                bc_pair = []
                for par in range(2):
                    bcp = ps.tile([128, 512], F32, tag="scp", bufs=4)
                    nc.tensor.matmul(
                        bcp[0:64, :], ones_sb[64:65, 0:64],
                        rc_pair[par][64:65, :], start=True, stop=True)
                    bc_pair.append(bcp)
                rr_pair = []
                for par in range(2):
                    rcs = sb.tile([64, 512], F32, tag="rcf", bufs=2)
                    nc.vector.reciprocal_approx_fast(
                        out=rcs[:], in_=bc_pair[par][0:64, :])
                    rr_pair.append(rcs)
                nc.vector.tensor_mul(
                    out=aot[0:64, 512 * st:512 * st + 512],
                    in0=av[0][0:64, :], in1=rr_pair[0][:])
                nc.vector.tensor_mul(
                    out=st2[:, 512 * st:512 * st + 512],
                    in0=av[1][0:64, :], in1=rr_pair[1][:])

            # ================= main schedule ==========================
            alloc_vaug(0)
            for b in range(BPC):
                qk = [emit_qkT(b, 0)]
                if b == 0:
                    # first batch's v-proj runs up front (dense warm-up);
                    # tt-outer so each v_aug tile completes early and the
                    # first head's AV matmuls aren't gated
                    for tt in range(NT):
                        for s in range(2):
                            emit_vproj_chain(0, s, tt)
                    alloc_vaug(1)
                    filler = [(emit_vproj_chain, (1, s, tt))
                              for tt in range(NT) for s in range(2)]
                else:
                    filler = [(emit_yproj_tt, (0, tt)) for tt in range(NT)]
                fi = 0

                attOutT[b] = []
                for hp in range(NP):
                    if hp + 1 < NP:
                        qk.append(emit_qkT(b, hp + 1))
                    qT, kT = qk[hp]
                    aot = sb.tile([128, T], BF16, tag="aot", bufs=2 * NP + 1)
                    attOutT[b].append(aot)
                    st2 = sb.tile([64, T], BF16, tag="stg2", bufs=2)

                    attn_st(b, hp, 0, qT, kT, aot, st2)
                    # par=1 st0 half -> aot rows 64:128 right away so
                    # st0-dependent y tiles can start
                    nc.sync.dma_start(aot[64:128, 0:512], st2[:, 0:512])
                    # interleave dense projection work of the other batch
                    n_fill = (len(filler) * (hp + 1) * 2 + NP * 2 - 1) \
                        // (NP * 2)
                    while fi < min(n_fill, len(filler)):
                        f, a = filler[fi]
                        f(*a)
                        fi += 1
                    if b == 1 and hp == NP - 1:
                        # y tiles 0-3 need only the st0 halves of every
                        # head: overlap them with the last head's st1
                        for tt in range(NT // 2):
                            emit_yproj_tt(1, tt)
                    attn_st(b, hp, 1, qT, kT, aot, st2)
                    nc.scalar.dma_start(aot[64:128, 512:1024],
                                        st2[:, 512:1024])
                    n_fill = (len(filler) * ((hp + 1) * 2 + 1)
                              + NP * 2 - 1) // (NP * 2)
                    while fi < min(n_fill, len(filler)):
                        f, a = filler[fi]
                        f(*a)
                        fi += 1
                while fi < len(filler):
                    f, a = filler[fi]
                    f(*a)
                    fi += 1
            # final batch's y projection tail (st1-dependent tiles)
            for tt in range(NT // 2, NT):
                emit_yproj_tt(1, tt)

    nc.finalize()
    return nc


def _prep_const_inputs(W_attn, b_attn, W_proj, b_proj):
    import ml_dtypes
    BF = ml_dtypes.bfloat16
    W_attn = np.asarray(W_attn, dtype=np.float32)
    W_proj = np.asarray(W_proj, dtype=np.float32)
    b_attn = np.asarray(b_attn, dtype=np.float32)
    b_proj = np.asarray(b_proj, dtype=np.float32)
    # wa packed: col block kc = wa[128*kc:128*(kc+1), :]  -> [128, NKC*3C]
    wa = np.ascontiguousarray(
        W_attn.reshape(NKC, 128, 3 * C).transpose(1, 0, 2).reshape(
            128, NKC * 3 * C)).astype(BF)
    wp = np.ascontiguousarray(
        W_proj.reshape(NP, 128, C).transpose(1, 0, 2).reshape(
            128, NP * C)).astype(BF)
    bqk = np.ascontiguousarray(
        b_attn[:2 * C].reshape(2 * NP, 128).T).astype(np.float32)
    bv = np.ascontiguousarray(
        np.broadcast_to(b_attn[2 * C:], (128, C))).astype(BF)
    bp = np.ascontiguousarray(
        np.broadcast_to(b_proj, (128, C))).astype(BF)
    # diag-tile mask: mask[i, j] = 1 if j >= i  (q-col >= k-row)
    jj = np.arange(128)[None, :]
    ii = np.arange(128)[:, None]
    mask = np.concatenate([(jj >= ii)] * 2, axis=1).astype(BF)
    ones = np.ones((128, 64), dtype=BF)
    return {"wa": wa, "wp": wp, "bqk": bqk, "bv": bv, "bp": bp,
            "mask": mask, "ones": ones}


def _make_in_maps(x, W_attn, b_attn, W_proj, b_proj):
    import ml_dtypes
    BF = ml_dtypes.bfloat16
    consts = _prep_const_inputs(W_attn, b_attn, W_proj, b_proj)
    xb = np.asarray(x, dtype=np.float32).astype(BF)
    maps = []
    for c in range(NCORES):
        xc = xb[BPC * c:BPC * (c + 1)]            # [BPC, T, C]
        xtc = np.ascontiguousarray(
            xc.transpose(2, 0, 1).reshape(C, BPC * T))
        maps.append({"xt": xtc, **consts})
    return maps


def kernel(x, W_attn, b_attn, W_proj, b_proj):
    from concourse.bass_utils import run_bass_kernel_spmd

    if "nc" not in _CACHE:
        _CACHE["nc"] = _build_nc()
    nc = _CACHE["nc"]

    in_maps = _make_in_maps(x, W_attn, b_attn, W_proj, b_proj)
    res = run_bass_kernel_spmd(nc, in_maps, list(range(NCORES)))
    return np.concatenate(
        [np.asarray(r["y"], dtype=np.float32) for r in res.results], axis=0)
